# revision 4
# baseline (speedup 1.0000x reference)
"""Bahdanau attention (B=32, S=2048, ENC2=1024, ATT=512) on 8 TRN2
NeuronCores, data-parallel over batch (4 batches/core), weights replicated.

v2: Uh computed in [s, a] layout (s on partitions) so that
  - the Ws bias lands via a Pool-engine PSUM preload (no PE energy matmuls,
    no per-partition-bias constraint),
  - the energy reduction v.tanh runs on DVE as one fused
    tensor_tensor_reduce per [128,512] tile,
  - softmax works on [128,16] tiles (128 s per partition-lane) instead of
    single-partition [1,2048] rows.
enc is converted f32->bf16 on ACT before the PE transposes (bf16 transpose
is 1.0 cycles/row vs 1.5 for f32r), and the Uh matmul runs in bf16 (same
1 cycle/row as f32r, ~0.4% rel err budget).

Per-core engine budget (TimelineSim): PE 141us (27.3 transpose + 109.2
matmul + prologue), ACT ~114us (conv c2c3 + Ws preloads + tanh), DVE
~107us (evacuations + energy mult/sum), Pool ~49us (conv c0/c1), DMA
~105us. TimelineSim total 165.2us; HW sync-slope ~166-185us/rep vs
baseline 213us.
"""

import numpy as np

import concourse.bass as bass
import concourse.mybir as mybir
import concourse.tile as tile
from concourse import bacc
from concourse.masks import make_identity

F32 = mybir.dt.float32
F32R = mybir.dt.float32r
BF16 = mybir.dt.bfloat16

N_CORES = 8
B_FULL, S, E, A = 32, 2048, 1024, 512
B_SH = B_FULL // N_CORES          # 4 batches per core
SBLK = 512                        # s-block
N_SBLK = S // SBLK                # 4 per batch
EJ = E // 128                     # 8 e-chunks
CC = SBLK // 128                  # 4 s-subchunks per s-block
NG = S // 128                     # 16 energy columns per batch


def r(ap):
    return ap.bitcast(F32R)


def build_program(reps=1, preload_mode="copy", pool_conv=True, use_ttr=False):
    nc = bacc.Bacc("TRN2", target_bir_lowering=False, debug=False,
                   num_devices=N_CORES)

    dec = nc.dram_tensor("decoder_hidden", [B_SH, E], F32R, kind="ExternalInput")
    enc = nc.dram_tensor("encoder_all_hidden", [B_SH, S, E], F32R,
                         kind="ExternalInput")
    W_w = nc.dram_tensor("W_w", [E, A], F32R, kind="ExternalInput")
    W_b = nc.dram_tensor("W_b", [A], F32R, kind="ExternalInput")
    U_w = nc.dram_tensor("U_w", [E, A], F32R, kind="ExternalInput")
    U_b = nc.dram_tensor("U_b", [A], F32R, kind="ExternalInput")
    v_w = nc.dram_tensor("v_w", [A, 1], F32R, kind="ExternalInput")
    alpha = nc.dram_tensor("alpha", [B_SH, S], F32, kind="ExternalOutput")

    with tile.TileContext(nc) as tc:
        with (
            tc.tile_pool(name="const", bufs=1) as constp,
            tc.tile_pool(name="x4", bufs=2) as x4p,
            tc.tile_pool(name="xbf", bufs=2) as xbfp,
            tc.tile_pool(name="enct", bufs=8) as enctp,
            tc.tile_pool(name="tanh", bufs=6) as tanhp,
            tc.tile_pool(name="scr", bufs=2) as scrp,
            tc.tile_pool(name="epi", bufs=2) as epip,
            tc.tile_pool(name="psT", bufs=4, space="PSUM") as psTp,
            tc.tile_pool(name="psU", bufs=3, space="PSUM") as psUp,
            tc.tile_pool(name="psE", bufs=1, space="PSUM") as psEp,
        ):
            # ---------------- prologue ----------------
            ident_f32 = constp.tile([128, 128], F32, tag="identf")
            make_identity(nc, ident_f32)
            ident = constp.tile([128, 128], F32R, tag="ident")
            nc.vector.tensor_copy(ident, ident_f32)
            ident_bf = constp.tile([128, 128], BF16, tag="identb")
            nc.vector.tensor_copy(ident_bf, ident_f32)

            # small loads on the ACT hwdge queue so they don't delay the
            # big SP-queue loads (each DMA instr costs ~0.65us dispatch)
            dec_sb = constp.tile([B_SH, E], F32R, tag="dec")
            nc.scalar.dma_start(dec_sb, dec[:, :])
            wb_sb = constp.tile([1, A], F32R, tag="wb")
            nc.scalar.dma_start(wb_sb, W_b[None, :])
            ub_sb = constp.tile([1, A], F32R, tag="ub")
            nc.scalar.dma_start(ub_sb, U_b[None, :])
            vrow = constp.tile([1, A], F32R, tag="v")
            nc.scalar.dma_start(vrow, v_w.rearrange("a o -> o a"))

            # dec transposes: PE work available immediately; all 8 land in
            # one psum tile, single DVE evac
            dect = constp.tile([128, B_SH * EJ], F32R, tag="dect")
            psd = psEp.tile([128, B_SH * EJ], F32R, tag="psE", name="psdec")
            for j in range(EJ):
                nc.tensor.transpose(r(psd[:, B_SH * j:B_SH * (j + 1)]),
                                    r(dec_sb[:, 128 * j:128 * (j + 1)]),
                                    r(ident[:B_SH, :B_SH]))
            nc.vector.tensor_copy(dect, psd)

            # DMA order tuned for the startup pipeline: first enc chunks
            # and U_w (feed transposes + matmuls) before W_w (whose Ws
            # contribution is deferred via a rank-1 matmul)
            x4_first = x4p.tile([128, CC * E], F32R, tag="x4")
            for c in range(2):
                nc.sync.dma_start(x4_first[:, E * c:E * (c + 1)],
                                  enc[0, 128 * c:128 * (c + 1), :])
            uw_f32 = constp.tile([128, EJ * A], F32R, tag="uwf")
            for j in range(EJ):
                nc.sync.dma_start(uw_f32[:, A * j:A * (j + 1)],
                                  U_w[128 * j:128 * (j + 1), :])
            for c in range(2, CC):
                nc.sync.dma_start(x4_first[:, E * c:E * (c + 1)],
                                  enc[0, 128 * c:128 * (c + 1), :])
            ww = constp.tile([128, EJ * A], F32R, tag="ww")
            nc.sync.dma_start(ww.rearrange("e (j a) -> e j a", j=EJ),
                              W_w.rearrange("(j e) a -> e j a", e=128))

            # U_w -> bf16 (DVE), emitted inside the first main-loop
            # iteration so the DVE queue interleaves with enc conversion
            uw_bf = constp.tile([128, EJ * A], BF16, tag="uwb")

            ones1f = constp.tile([1, 128], F32, tag="ones1f")
            nc.vector.memset(ones1f, 1.0)
            ones1 = constp.tile([1, 128], F32R, tag="ones1")
            nc.vector.tensor_copy(ones1, ones1f)
            # remaining small constants are built later (inside the Ws
            # chain) so they don't clutter the startup queue heads
            bias_sum = constp.tile([1, A], F32R, tag="bias")
            ones14 = constp.tile([1, B_SH], F32R, tag="ones")
            onesc = constp.tile([128, 4], F32R, tag="onesc")

            # v broadcast [128, A] bf16 via PE ones-outer-product
            ps_v = psEp.tile([128, A], F32, tag="psE", name="ps_v")
            nc.tensor.matmul(ps_v, r(ones1), r(vrow), start=True, stop=True)
            v_bc = constp.tile([128, A], BF16, tag="vbc")
            nc.scalar.copy(v_bc, ps_v)

            ws_bc = constp.tile([128, B_SH * A], F32R, tag="wsbc")

            sel = constp.tile([B_SH, B_SH * 128], F32R, tag="sel")

            ws_sb = constp.tile([B_SH, A], F32R, tag="ws", name="ws_sb")

            def prologue_ws_chain():
                ones14f = constp.tile([1, B_SH], F32, tag="onesf")
                nc.vector.memset(ones14f, 1.0)
                nc.vector.tensor_copy(ones14, ones14f)
                onescf = constp.tile([128, 4], F32, tag="onescf")
                nc.vector.memset(onescf, 1.0)
                nc.vector.tensor_copy(onesc, onescf)
                nc.vector.tensor_tensor(out=bias_sum, in0=wb_sb, in1=ub_sb,
                                        op=mybir.AluOpType.add)
                # selector stationaries: sel[:, 128b:...][k, m] = (k == b),
                # to broadcast row b of ws_sb over all 128 partitions
                self_f = constp.tile([B_SH, B_SH * 128], F32, tag="selp")
                nc.gpsimd.memset(self_f, 0.0)
                nc.gpsimd.affine_select(
                    out=self_f, in_=self_f,
                    compare_op=mybir.AluOpType.not_equal,
                    fill=1.0, base=0,
                    pattern=[[-1, B_SH], [0, 128]],
                    channel_multiplier=1)
                nc.vector.tensor_copy(sel, self_f)
                # Ws = dec @ W_w + (W_b + U_b): psum [B_SH, A]
                ps_ws = psEp.tile([B_SH, A], F32, tag="psE", name="ps_ws")
                for j in range(EJ):
                    nc.tensor.matmul(ps_ws,
                                     r(dect[:, B_SH * j:B_SH * (j + 1)]),
                                     r(ww[:, A * j:A * (j + 1)]),
                                     start=(j == 0), stop=False)
                nc.tensor.matmul(ps_ws, r(ones14), r(bias_sum),
                                 start=False, stop=True)
                nc.scalar.copy(ws_sb, ps_ws)

            def prologue_ws_bcast():
                # broadcast each Ws row over 128 partitions (PE selector
                # matmuls, ACT evacs)
                for b in range(B_SH):
                    ps_w = psEp.tile([128, A], F32, tag="psE",
                                     name=f"ps_wbc_{b}")
                    nc.tensor.matmul(ps_w, sel[:, 128 * b:128 * (b + 1)],
                                     ws_sb, start=True, stop=True)
                    nc.scalar.copy(ws_bc[:, A * b:A * (b + 1)], ps_w)

            # ---------------- main loop ----------------
            pending_epi = []
            deferred_tail = []  # deferred tanh (ACT) closures
            deferred_ttr = []   # deferred TTR (DVE) closures

            def emit_epilogue(pending_epi):
                # f32r intermediates: the den/broadcast/transpose matmuls
                # need f32r-produced operands (BIR verifier); the f32r
                # rounding is far inside the 2e-2 tolerance
                b, rep, energy_b = pending_epi.pop(0)
                exp_all = epip.tile([128, NG], F32, tag="exp",
                                    name=f"exp_{rep}_{b}")
                rowsum = epip.tile([128, 1], F32R, tag="rsum",
                                   name=f"rsum_{rep}_{b}")
                with nc.allow_low_precision("f32r accum for PE consumer"):
                    nc.scalar.activation(
                        out=exp_all, in_=energy_b,
                        func=mybir.ActivationFunctionType.Exp,
                        accum_out=rowsum)
                # N=4 matmuls (fp32r requires moving free >= 4); the den
                # and its reciprocal come out replicated 4x
                ps_den = psEp.tile([1, 4], F32, tag="psE",
                                   name=f"psden_{rep}_{b}")
                nc.tensor.matmul(ps_den, rowsum, onesc,
                                 start=True, stop=True)
                rec_sb = epip.tile([1, 4], F32R, tag="rec",
                                   name=f"rec_{rep}_{b}")
                with nc.allow_low_precision("f32r recip for PE consumer"):
                    nc.vector.reciprocal(rec_sb, ps_den)
                ps_rb = psEp.tile([128, 4], F32, tag="psE",
                                  name=f"psrb_{rep}_{b}")
                nc.tensor.matmul(ps_rb, r(ones1), rec_sb,
                                 start=True, stop=True)
                rec128 = epip.tile([128, 1], F32, tag="rec128",
                                   name=f"rec128_{rep}_{b}")
                nc.vector.tensor_copy(rec128, ps_rb[:, 0:1])
                alpha_sc = epip.tile([128, NG], F32R, tag="asc",
                                     name=f"asc_{rep}_{b}")
                nc.vector.tensor_scalar_mul(alpha_sc, exp_all, rec128)
                ps_al = psEp.tile([NG, 128], F32R, tag="psE",
                                  name=f"psal_{rep}_{b}")
                nc.tensor.transpose(ps_al, alpha_sc, r(ident))
                al_sb = epip.tile([NG, 128], F32, tag="al",
                                  name=f"al_{rep}_{b}")
                nc.vector.tensor_copy(al_sb, ps_al.bitcast(F32))
                nc.sync.dma_start(
                    alpha[b:b + 1, :].rearrange("o (c s) -> (o c) s", c=NG),
                    al_sb)

            for rep in range(reps):
              for b in range(B_SH):
                energy_b = epip.tile([128, NG], F32, tag="en",
                                     name=f"en_{rep}_{b}")
                for sblk in range(N_SBLK):
                    s0 = SBLK * sblk
                    first = (b == 0 and sblk == 0 and rep == 0)
                    if first:
                        x4 = x4_first
                    else:
                        x4 = x4p.tile([128, CC * E], F32R, tag="x4")
                        nc.sync.dma_start(
                            x4.rearrange("p (c e) -> p c e", c=CC),
                            enc[b, s0:s0 + SBLK, :]
                            .rearrange("(c p) e -> p c e", p=128))

                    last_iter = (rep == reps - 1 and b == B_SH - 1
                                 and sblk == N_SBLK - 1)
                    xbf = xbfp.tile([128, CC * E], BF16, tag="xbf")
                    # per-c psum tiles (4-deep ring) and per-c sbuf
                    # destinations: the tile framework tracks dependencies
                    # at tile granularity, so sharing one wide tile would
                    # serialize transposes behind evacuations
                    enct = [None] * CC
                    psts = [None] * CC
                    ps_uhs = [None] * CC

                    def emit_T(c, xbf=xbf, psts=psts, rep=rep, b=b,
                               sblk=sblk):
                        pst = psTp.tile([128, EJ * 128], BF16, tag="psT")
                        for j in range(EJ):
                            nc.tensor.transpose(
                                pst[:, 128 * j:128 * (j + 1)],
                                xbf[:, E * c + 128 * j:E * c + 128 * (j + 1)],
                                ident_bf)
                        psts[c] = pst

                    def emit_evac(c, enct=enct, psts=psts, rep=rep, b=b,
                                  sblk=sblk):
                        ec = enctp.tile([128, EJ * 128], BF16, tag="enct",
                                        name=f"enct_{rep}_{b}_{sblk}_{c}")
                        nc.vector.tensor_copy(ec, psts[c])
                        enct[c] = ec

                    def emit_pre(c, engine, ps_uhs=ps_uhs, b=b):
                        # seed PSUM with the Ws broadcast (GPSIMD cannot
                        # access PSUM, so ACT or DVE)
                        ps_uh = psUp.tile([128, A], F32, tag="psU")
                        if engine == "act":
                            nc.scalar.copy(ps_uh, ws_bc[:, A * b:A * (b + 1)])
                        else:
                            nc.vector.tensor_copy(
                                ps_uh, ws_bc[:, A * b:A * (b + 1)])
                        ps_uhs[c] = ps_uh

                    def emit_M(c, preload, enct=enct, ps_uhs=ps_uhs, b=b):
                        # preload: None -> first-block mode (group closed
                        # later by a rank-1 ones x Ws matmul); 'act'/'dve'
                        # -> seed now; 'done' -> already seeded
                        if preload is not None and preload_mode == "ones":
                            preload = None
                        if preload is None:
                            ps_uh = psUp.tile([128, A], F32, tag="psU")
                            ps_uhs[c] = ps_uh
                        elif preload != "done":
                            emit_pre(c, preload)
                        ps_uh = ps_uhs[c]
                        for j in range(EJ):
                            nc.tensor.matmul(
                                ps_uh,
                                enct[c][:, 128 * j:128 * (j + 1)],
                                uw_bf[:, A * j:A * (j + 1)],
                                start=(preload is None and j == 0),
                                stop=(preload is not None and j == EJ - 1),
                                skip_group_check=True)
                        if preload is None and not first:
                            # Ws via rank-1 ones x Ws broadcast row
                            nc.tensor.matmul(
                                ps_uh, r(ones1),
                                ws_bc[0:1, A * b:A * (b + 1)],
                                start=False, stop=True,
                                skip_group_check=True)

                    def mk_tail(c, rep=rep, b=b, sblk=sblk,
                                energy_b=energy_b, ps_uhs=ps_uhs):
                        th = tanhp.tile([128, A], BF16, tag="tanh",
                                        name=f"tanh_{rep}_{b}_{sblk}_{c}")

                        def tail_tanh():
                            nc.scalar.activation(
                                out=th, in_=ps_uhs[c],
                                func=mybir.ActivationFunctionType.Tanh)

                        def tail_ttr():
                            scr = scrp.tile([128, A], BF16, tag="scr")
                            g = CC * sblk + c
                            if use_ttr:
                                nc.vector.tensor_tensor_reduce(
                                    out=scr, in0=th, in1=v_bc,
                                    scale=1.0, scalar=0.0,
                                    op0=mybir.AluOpType.mult,
                                    op1=mybir.AluOpType.add,
                                    accum_out=energy_b[:, g:g + 1])
                            else:
                                # v*tanh (bf16 2x) + free-dim sum, both DVE
                                nc.vector.tensor_tensor(
                                    out=scr, in0=th, in1=v_bc,
                                    op=mybir.AluOpType.mult)
                                nc.vector.reduce_sum(
                                    energy_b[:, g:g + 1], scr,
                                    axis=mybir.AxisListType.X)

                        return tail_tanh, tail_ttr

                    def emit_tails(c):
                        tail_tanh, tail_ttr = mk_tail(c)
                        if c == CC - 1 and not last_iter:
                            deferred_tail.append(tail_tanh)
                            deferred_ttr.append(tail_ttr)
                        else:
                            tail_tanh()
                            tail_ttr()

                    if first:
                        # interleaved startup emission: each PE stage's
                        # input arrives just-in-time from the DMA order
                        # (x4c0, x4c1, U_w, x4c2, x4c3, W_w); the Ws
                        # contribution is added by a rank-1 matmul once
                        # W_w lands, instead of the Pool preload
                        if pool_conv:
                            nc.gpsimd.tensor_copy(xbf[:, :E], x4[:, :E])
                        else:
                            nc.scalar.copy(xbf[:, :E], x4[:, :E])
                        for j in range(4):
                            nc.vector.tensor_copy(uw_bf[:, A * j:A * (j + 1)],
                                                  uw_f32[:, A * j:A * (j + 1)])
                        nc.scalar.copy(xbf[:, E:2 * E], x4[:, E:2 * E])
                        emit_T(0)
                        emit_evac(0)
                        emit_T(1)
                        emit_evac(1)
                        nc.scalar.copy(xbf[:, 2 * E:3 * E], x4[:, 2 * E:3 * E])
                        for j in range(4, EJ):
                            nc.vector.tensor_copy(uw_bf[:, A * j:A * (j + 1)],
                                                  uw_f32[:, A * j:A * (j + 1)])
                        emit_M(0, preload=None)
                        emit_T(2)
                        emit_evac(2)
                        emit_M(1, preload=None)
                        nc.scalar.copy(xbf[:, 3 * E:], x4[:, 3 * E:])
                        emit_T(3)
                        emit_evac(3)
                        emit_M(2, preload=None)
                        emit_M(3, preload=None)
                        prologue_ws_chain()
                        prologue_ws_bcast()
                        for c in range(CC):
                            # close each psum group with ones x Ws[0]
                            nc.tensor.matmul(ps_uhs[c], r(ones1),
                                             r(ws_sb[0:1, :]),
                                             start=False, stop=True,
                                             skip_group_check=True)
                        for c in range(CC):
                            emit_tails(c)
                    else:
                        # steady state: conv c0+c1 on Pool, c2c3 as one
                        # wide ACT op; Ws preloads c0/c1 on ACT (ahead of
                        # the tanhs), c2/c3 on DVE; energy mult+sum on DVE
                        if pool_conv:
                            nc.gpsimd.tensor_copy(xbf[:, :E], x4[:, :E])
                            nc.gpsimd.tensor_copy(xbf[:, E:2 * E],
                                                  x4[:, E:2 * E])
                        else:
                            nc.scalar.copy(xbf[:, :E], x4[:, :E])
                            nc.scalar.copy(xbf[:, E:2 * E], x4[:, E:2 * E])
                        nc.scalar.copy(xbf[:, 2 * E:], x4[:, 2 * E:])
                        if preload_mode != "ones":
                            emit_pre(0, "act")
                            emit_pre(1, "act")

                        # tanh deferred from the previous iteration's c3:
                        # emitted after this block's conversion so the
                        # ACT queue doesn't convoy behind the PE's last
                        # matmul; its mult+sum after the c1 evacuation
                        if deferred_tail:
                            deferred_tail.pop(0)()
                        emit_T(0)
                        emit_evac(0)
                        emit_T(1)
                        emit_evac(1)
                        if deferred_ttr:
                            deferred_ttr.pop(0)()
                        emit_T(2)
                        emit_evac(2)
                        emit_T(3)
                        emit_evac(3)
                        if preload_mode == "ones":
                            for c in range(CC):
                                emit_M(c, preload="ones")
                                emit_tails(c)
                        else:
                            emit_M(0, preload="done")
                            emit_M(1, preload="done")
                            emit_M(2, preload="act")
                            emit_tails(0)
                            emit_M(3, preload="act")
                            emit_tails(1)
                            emit_tails(2)
                            emit_tails(3)

                    # epilogue for the previous batch goes after this
                    # sblk's PE stream so PE never waits on it
                    if sblk == 0 and pending_epi:
                        emit_epilogue(pending_epi)

                pending_epi.append((b, rep, energy_b))
                if b == B_SH - 1:
                    emit_epilogue(pending_epi)

    nc.compile()
    return nc


def shard_inputs(inputs):
    """Full inputs dict -> list of 8 per-core input dicts."""
    dec = np.ascontiguousarray(inputs["decoder_hidden"], dtype=np.float32)
    enc = np.ascontiguousarray(inputs["encoder_all_hidden"], dtype=np.float32)
    base = {
        "W_w": np.ascontiguousarray(inputs["W_w"], dtype=np.float32),
        "W_b": np.ascontiguousarray(inputs["W_b"], dtype=np.float32),
        "U_w": np.ascontiguousarray(inputs["U_w"], dtype=np.float32),
        "U_b": np.ascontiguousarray(inputs["U_b"], dtype=np.float32),
        "v_w": np.ascontiguousarray(inputs["v_w"], dtype=np.float32),
    }
    maps = []
    for c in range(N_CORES):
        m = dict(base)
        m["decoder_hidden"] = dec[c * B_SH:(c + 1) * B_SH]
        m["encoder_all_hidden"] = enc[c * B_SH:(c + 1) * B_SH]
        maps.append(m)
    return maps


_NC_CACHE = None


def get_program():
    global _NC_CACHE
    if _NC_CACHE is None:
        _NC_CACHE = build_program()
    return _NC_CACHE


def kernel(**inputs):
    from concourse import bass_utils
    nc = get_program()
    maps = shard_inputs(inputs)
    res = bass_utils.run_bass_kernel_spmd(nc, maps,
                                          core_ids=list(range(N_CORES)))
    return np.concatenate([res.results[c]["alpha"] for c in range(N_CORES)],
                          axis=0)


# revision 5
# speedup vs baseline: 1.1821x; 1.1821x over previous
"""Bahdanau attention (B=32, S=2048, ENC2=1024, ATT=512) on 8 TRN2
NeuronCores, data-parallel over batch (4 batches/core), weights replicated.

v2: Uh computed in [s, a] layout (s on partitions) so that
  - the Ws bias lands via a Pool-engine PSUM preload (no PE energy matmuls,
    no per-partition-bias constraint),
  - the energy reduction v.tanh runs on DVE as one fused
    tensor_tensor_reduce per [128,512] tile,
  - softmax works on [128,16] tiles (128 s per partition-lane) instead of
    single-partition [1,2048] rows.
enc is converted f32->bf16 on ACT before the PE transposes (bf16 transpose
is 1.0 cycles/row vs 1.5 for f32r), and the Uh matmul runs in bf16 (same
1 cycle/row as f32r, ~0.4% rel err budget).

Per-core engine budget (TimelineSim): PE 141us (27.3 transpose + 109.2
matmul + prologue), ACT ~114us (conv c2c3 + Ws preloads + tanh), DVE
~107us (evacuations + energy mult/sum), Pool ~49us (conv c0/c1), DMA
~105us. TimelineSim total 165.2us; HW sync-slope ~166-185us/rep vs
baseline 213us.
"""

import numpy as np

import concourse.bass as bass
import concourse.mybir as mybir
import concourse.tile as tile
from concourse import bacc
from concourse.masks import make_identity

F32 = mybir.dt.float32
F32R = mybir.dt.float32r
BF16 = mybir.dt.bfloat16

N_CORES = 8
B_FULL, S, E, A = 32, 2048, 1024, 512
B_SH = B_FULL // N_CORES          # 4 batches per core
SBLK = 512                        # s-block
N_SBLK = S // SBLK                # 4 per batch
EJ = E // 128                     # 8 e-chunks
CC = SBLK // 128                  # 4 s-subchunks per s-block
NG = S // 128                     # 16 energy columns per batch


def r(ap):
    return ap.bitcast(F32R)


def build_program(reps=1, preload_mode="copy", pool_conv=False, use_ttr=False):
    nc = bacc.Bacc("TRN2", target_bir_lowering=False, debug=False,
                   num_devices=N_CORES)

    dec = nc.dram_tensor("decoder_hidden", [B_SH, E], F32R, kind="ExternalInput")
    enc = nc.dram_tensor("encoder_all_hidden", [B_SH, S, E], F32R,
                         kind="ExternalInput")
    W_w = nc.dram_tensor("W_w", [E, A], F32R, kind="ExternalInput")
    W_b = nc.dram_tensor("W_b", [A], F32R, kind="ExternalInput")
    U_w = nc.dram_tensor("U_w", [E, A], F32R, kind="ExternalInput")
    U_b = nc.dram_tensor("U_b", [A], F32R, kind="ExternalInput")
    v_w = nc.dram_tensor("v_w", [A, 1], F32R, kind="ExternalInput")
    alpha = nc.dram_tensor("alpha", [B_SH, S], F32, kind="ExternalOutput")

    with tile.TileContext(nc) as tc:
        with (
            tc.tile_pool(name="const", bufs=1) as constp,
            tc.tile_pool(name="x4", bufs=2) as x4p,
            tc.tile_pool(name="xbf", bufs=2) as xbfp,
            tc.tile_pool(name="enct", bufs=8) as enctp,
            tc.tile_pool(name="tanh", bufs=6) as tanhp,
            tc.tile_pool(name="scr", bufs=2) as scrp,
            tc.tile_pool(name="epi", bufs=2) as epip,
            tc.tile_pool(name="psT", bufs=4, space="PSUM") as psTp,
            tc.tile_pool(name="psU", bufs=3, space="PSUM") as psUp,
            tc.tile_pool(name="psE", bufs=1, space="PSUM") as psEp,
        ):
            # ---------------- prologue ----------------
            ident_f32 = constp.tile([128, 128], F32, tag="identf")
            make_identity(nc, ident_f32)
            ident = constp.tile([128, 128], F32R, tag="ident")
            nc.vector.tensor_copy(ident, ident_f32)
            ident_bf = constp.tile([128, 128], BF16, tag="identb")
            nc.vector.tensor_copy(ident_bf, ident_f32)

            # small loads on the ACT hwdge queue so they don't delay the
            # big SP-queue loads (each DMA instr costs ~0.65us dispatch)
            dec_sb = constp.tile([B_SH, E], F32R, tag="dec")
            nc.scalar.dma_start(dec_sb, dec[:, :])
            wb_sb = constp.tile([1, A], F32R, tag="wb")
            nc.scalar.dma_start(wb_sb, W_b[None, :])
            ub_sb = constp.tile([1, A], F32R, tag="ub")
            nc.scalar.dma_start(ub_sb, U_b[None, :])
            vrow = constp.tile([1, A], F32R, tag="v")
            nc.scalar.dma_start(vrow, v_w.rearrange("a o -> o a"))

            # dec transposes: PE work available immediately; all 8 land in
            # one psum tile, single DVE evac
            dect = constp.tile([128, B_SH * EJ], F32R, tag="dect")
            psd = psEp.tile([128, B_SH * EJ], F32R, tag="psE", name="psdec")
            for j in range(EJ):
                nc.tensor.transpose(r(psd[:, B_SH * j:B_SH * (j + 1)]),
                                    r(dec_sb[:, 128 * j:128 * (j + 1)]),
                                    r(ident[:B_SH, :B_SH]))
            nc.vector.tensor_copy(dect, psd)

            # DMA order tuned for the startup pipeline: first enc chunks
            # and U_w (feed transposes + matmuls) before W_w (whose Ws
            # contribution is deferred via a rank-1 matmul)
            x4_first = x4p.tile([128, CC * E], F32R, tag="x4")
            for c in range(2):
                nc.sync.dma_start(x4_first[:, E * c:E * (c + 1)],
                                  enc[0, 128 * c:128 * (c + 1), :])
            uw_f32 = constp.tile([128, EJ * A], F32R, tag="uwf")
            for j in range(EJ):
                nc.sync.dma_start(uw_f32[:, A * j:A * (j + 1)],
                                  U_w[128 * j:128 * (j + 1), :])
            for c in range(2, CC):
                nc.sync.dma_start(x4_first[:, E * c:E * (c + 1)],
                                  enc[0, 128 * c:128 * (c + 1), :])
            ww = constp.tile([128, EJ * A], F32R, tag="ww")
            nc.sync.dma_start(ww.rearrange("e (j a) -> e j a", j=EJ),
                              W_w.rearrange("(j e) a -> e j a", e=128))

            # U_w -> bf16 (DVE), emitted inside the first main-loop
            # iteration so the DVE queue interleaves with enc conversion
            uw_bf = constp.tile([128, EJ * A], BF16, tag="uwb")

            ones1f = constp.tile([1, 128], F32, tag="ones1f")
            nc.vector.memset(ones1f, 1.0)
            ones1 = constp.tile([1, 128], F32R, tag="ones1")
            nc.vector.tensor_copy(ones1, ones1f)
            # remaining small constants are built later (inside the Ws
            # chain) so they don't clutter the startup queue heads
            bias_sum = constp.tile([1, A], F32R, tag="bias")
            ones14 = constp.tile([1, B_SH], F32R, tag="ones")
            onesc = constp.tile([128, 4], F32R, tag="onesc")

            # v broadcast [128, A] bf16 via PE ones-outer-product
            ps_v = psEp.tile([128, A], F32, tag="psE", name="ps_v")
            nc.tensor.matmul(ps_v, r(ones1), r(vrow), start=True, stop=True)
            v_bc = constp.tile([128, A], BF16, tag="vbc")
            nc.scalar.copy(v_bc, ps_v)

            ws_bc = constp.tile([128, B_SH * A], F32R, tag="wsbc")

            sel = constp.tile([B_SH, B_SH * 128], F32R, tag="sel")

            ws_sb = constp.tile([B_SH, A], F32R, tag="ws", name="ws_sb")

            def prologue_ws_chain():
                ones14f = constp.tile([1, B_SH], F32, tag="onesf")
                nc.vector.memset(ones14f, 1.0)
                nc.vector.tensor_copy(ones14, ones14f)
                onescf = constp.tile([128, 4], F32, tag="onescf")
                nc.vector.memset(onescf, 1.0)
                nc.vector.tensor_copy(onesc, onescf)
                nc.vector.tensor_tensor(out=bias_sum, in0=wb_sb, in1=ub_sb,
                                        op=mybir.AluOpType.add)
                # selector stationaries: sel[:, 128b:...][k, m] = (k == b),
                # to broadcast row b of ws_sb over all 128 partitions
                self_f = constp.tile([B_SH, B_SH * 128], F32, tag="selp")
                nc.gpsimd.memset(self_f, 0.0)
                nc.gpsimd.affine_select(
                    out=self_f, in_=self_f,
                    compare_op=mybir.AluOpType.not_equal,
                    fill=1.0, base=0,
                    pattern=[[-1, B_SH], [0, 128]],
                    channel_multiplier=1)
                nc.vector.tensor_copy(sel, self_f)
                # Ws = dec @ W_w + (W_b + U_b): psum [B_SH, A]
                ps_ws = psEp.tile([B_SH, A], F32, tag="psE", name="ps_ws")
                for j in range(EJ):
                    nc.tensor.matmul(ps_ws,
                                     r(dect[:, B_SH * j:B_SH * (j + 1)]),
                                     r(ww[:, A * j:A * (j + 1)]),
                                     start=(j == 0), stop=False)
                nc.tensor.matmul(ps_ws, r(ones14), r(bias_sum),
                                 start=False, stop=True)
                nc.scalar.copy(ws_sb, ps_ws)

            def prologue_ws_bcast():
                # broadcast each Ws row over 128 partitions (PE selector
                # matmuls, ACT evacs)
                for b in range(B_SH):
                    ps_w = psEp.tile([128, A], F32, tag="psE",
                                     name=f"ps_wbc_{b}")
                    nc.tensor.matmul(ps_w, sel[:, 128 * b:128 * (b + 1)],
                                     ws_sb, start=True, stop=True)
                    nc.scalar.copy(ws_bc[:, A * b:A * (b + 1)], ps_w)

            # ---------------- main loop ----------------
            pending_epi = []
            deferred_tail = []  # deferred tanh (ACT) closures
            deferred_ttr = []   # deferred TTR (DVE) closures

            def emit_epilogue(pending_epi):
                # f32r intermediates: the den/broadcast/transpose matmuls
                # need f32r-produced operands (BIR verifier); the f32r
                # rounding is far inside the 2e-2 tolerance
                b, rep, energy_b = pending_epi.pop(0)
                exp_all = epip.tile([128, NG], F32, tag="exp",
                                    name=f"exp_{rep}_{b}")
                rowsum = epip.tile([128, 1], F32R, tag="rsum",
                                   name=f"rsum_{rep}_{b}")
                with nc.allow_low_precision("f32r accum for PE consumer"):
                    nc.scalar.activation(
                        out=exp_all, in_=energy_b,
                        func=mybir.ActivationFunctionType.Exp,
                        accum_out=rowsum)
                # N=4 matmuls (fp32r requires moving free >= 4); the den
                # and its reciprocal come out replicated 4x
                ps_den = psEp.tile([1, 4], F32, tag="psE",
                                   name=f"psden_{rep}_{b}")
                nc.tensor.matmul(ps_den, rowsum, onesc,
                                 start=True, stop=True)
                rec_sb = epip.tile([1, 4], F32R, tag="rec",
                                   name=f"rec_{rep}_{b}")
                with nc.allow_low_precision("f32r recip for PE consumer"):
                    nc.vector.reciprocal(rec_sb, ps_den)
                ps_rb = psEp.tile([128, 4], F32, tag="psE",
                                  name=f"psrb_{rep}_{b}")
                nc.tensor.matmul(ps_rb, r(ones1), rec_sb,
                                 start=True, stop=True)
                rec128 = epip.tile([128, 1], F32, tag="rec128",
                                   name=f"rec128_{rep}_{b}")
                nc.vector.tensor_copy(rec128, ps_rb[:, 0:1])
                alpha_sc = epip.tile([128, NG], F32R, tag="asc",
                                     name=f"asc_{rep}_{b}")
                nc.vector.tensor_scalar_mul(alpha_sc, exp_all, rec128)
                ps_al = psEp.tile([NG, 128], F32R, tag="psE",
                                  name=f"psal_{rep}_{b}")
                nc.tensor.transpose(ps_al, alpha_sc, r(ident))
                al_sb = epip.tile([NG, 128], F32, tag="al",
                                  name=f"al_{rep}_{b}")
                nc.vector.tensor_copy(al_sb, ps_al.bitcast(F32))
                nc.sync.dma_start(
                    alpha[b:b + 1, :].rearrange("o (c s) -> (o c) s", c=NG),
                    al_sb)

            for rep in range(reps):
              for b in range(B_SH):
                energy_b = epip.tile([128, NG], F32, tag="en",
                                     name=f"en_{rep}_{b}")
                for sblk in range(N_SBLK):
                    s0 = SBLK * sblk
                    first = (b == 0 and sblk == 0 and rep == 0)
                    if first:
                        x4 = x4_first
                    else:
                        x4 = x4p.tile([128, CC * E], F32R, tag="x4")
                        nc.sync.dma_start(
                            x4.rearrange("p (c e) -> p c e", c=CC),
                            enc[b, s0:s0 + SBLK, :]
                            .rearrange("(c p) e -> p c e", p=128))

                    last_iter = (rep == reps - 1 and b == B_SH - 1
                                 and sblk == N_SBLK - 1)
                    xbf = xbfp.tile([128, CC * E], BF16, tag="xbf")
                    # per-c psum tiles (4-deep ring) and per-c sbuf
                    # destinations: the tile framework tracks dependencies
                    # at tile granularity, so sharing one wide tile would
                    # serialize transposes behind evacuations
                    enct = [None] * CC
                    psts = [None] * CC
                    ps_uhs = [None] * CC

                    def emit_T(c, xbf=xbf, psts=psts, rep=rep, b=b,
                               sblk=sblk):
                        pst = psTp.tile([128, EJ * 128], BF16, tag="psT")
                        for j in range(EJ):
                            nc.tensor.transpose(
                                pst[:, 128 * j:128 * (j + 1)],
                                xbf[:, E * c + 128 * j:E * c + 128 * (j + 1)],
                                ident_bf)
                        psts[c] = pst

                    def emit_evac(c, enct=enct, psts=psts, rep=rep, b=b,
                                  sblk=sblk):
                        ec = enctp.tile([128, EJ * 128], BF16, tag="enct",
                                        name=f"enct_{rep}_{b}_{sblk}_{c}")
                        nc.vector.tensor_copy(ec, psts[c])
                        enct[c] = ec

                    def emit_pre(c, engine, ps_uhs=ps_uhs, b=b):
                        # seed PSUM with the Ws broadcast (GPSIMD cannot
                        # access PSUM, so ACT or DVE)
                        ps_uh = psUp.tile([128, A], F32, tag="psU")
                        if engine == "act":
                            nc.scalar.copy(ps_uh, ws_bc[:, A * b:A * (b + 1)])
                        else:
                            nc.vector.tensor_copy(
                                ps_uh, ws_bc[:, A * b:A * (b + 1)])
                        ps_uhs[c] = ps_uh

                    def emit_M(c, preload, enct=enct, ps_uhs=ps_uhs, b=b):
                        # preload: None -> first-block mode (group closed
                        # later by a rank-1 ones x Ws matmul); 'act'/'dve'
                        # -> seed now; 'done' -> already seeded
                        if preload is not None and preload_mode == "ones":
                            preload = None
                        if preload is None:
                            ps_uh = psUp.tile([128, A], F32, tag="psU")
                            ps_uhs[c] = ps_uh
                        elif preload != "done":
                            emit_pre(c, preload)
                        ps_uh = ps_uhs[c]
                        for j in range(EJ):
                            nc.tensor.matmul(
                                ps_uh,
                                enct[c][:, 128 * j:128 * (j + 1)],
                                uw_bf[:, A * j:A * (j + 1)],
                                start=(preload is None and j == 0),
                                stop=(preload is not None and j == EJ - 1),
                                skip_group_check=True)
                        if preload is None and not first:
                            # Ws via rank-1 ones x Ws broadcast row
                            nc.tensor.matmul(
                                ps_uh, r(ones1),
                                ws_bc[0:1, A * b:A * (b + 1)],
                                start=False, stop=True,
                                skip_group_check=True)

                    def mk_tail(c, rep=rep, b=b, sblk=sblk,
                                energy_b=energy_b, ps_uhs=ps_uhs):
                        th = tanhp.tile([128, A], BF16, tag="tanh",
                                        name=f"tanh_{rep}_{b}_{sblk}_{c}")

                        def tail_tanh():
                            nc.scalar.activation(
                                out=th, in_=ps_uhs[c],
                                func=mybir.ActivationFunctionType.Tanh)

                        def tail_ttr():
                            scr = scrp.tile([128, A], BF16, tag="scr")
                            g = CC * sblk + c
                            if use_ttr:
                                nc.vector.tensor_tensor_reduce(
                                    out=scr, in0=th, in1=v_bc,
                                    scale=1.0, scalar=0.0,
                                    op0=mybir.AluOpType.mult,
                                    op1=mybir.AluOpType.add,
                                    accum_out=energy_b[:, g:g + 1])
                            else:
                                # v*tanh (bf16 2x) + free-dim sum, both DVE
                                nc.vector.tensor_tensor(
                                    out=scr, in0=th, in1=v_bc,
                                    op=mybir.AluOpType.mult)
                                nc.vector.reduce_sum(
                                    energy_b[:, g:g + 1], scr,
                                    axis=mybir.AxisListType.X)

                        return tail_tanh, tail_ttr

                    def emit_tails(c):
                        tail_tanh, tail_ttr = mk_tail(c)
                        if c == CC - 1 and not last_iter:
                            deferred_tail.append(tail_tanh)
                            deferred_ttr.append(tail_ttr)
                        else:
                            tail_tanh()
                            tail_ttr()

                    if first:
                        # interleaved startup emission: each PE stage's
                        # input arrives just-in-time from the DMA order
                        # (x4c0, x4c1, U_w, x4c2, x4c3, W_w); the Ws
                        # contribution is added by a rank-1 matmul once
                        # W_w lands, instead of the Pool preload
                        if pool_conv:
                            nc.gpsimd.tensor_copy(xbf[:, :E], x4[:, :E])
                        else:
                            nc.scalar.copy(xbf[:, :E], x4[:, :E])
                        for j in range(4):
                            nc.vector.tensor_copy(uw_bf[:, A * j:A * (j + 1)],
                                                  uw_f32[:, A * j:A * (j + 1)])
                        nc.scalar.copy(xbf[:, E:2 * E], x4[:, E:2 * E])
                        emit_T(0)
                        emit_evac(0)
                        emit_T(1)
                        emit_evac(1)
                        nc.scalar.copy(xbf[:, 2 * E:3 * E], x4[:, 2 * E:3 * E])
                        for j in range(4, EJ):
                            nc.vector.tensor_copy(uw_bf[:, A * j:A * (j + 1)],
                                                  uw_f32[:, A * j:A * (j + 1)])
                        emit_M(0, preload=None)
                        emit_T(2)
                        emit_evac(2)
                        emit_M(1, preload=None)
                        nc.scalar.copy(xbf[:, 3 * E:], x4[:, 3 * E:])
                        emit_T(3)
                        emit_evac(3)
                        emit_M(2, preload=None)
                        emit_M(3, preload=None)
                        prologue_ws_chain()
                        prologue_ws_bcast()
                        for c in range(CC):
                            # close each psum group with ones x Ws[0]
                            nc.tensor.matmul(ps_uhs[c], r(ones1),
                                             r(ws_sb[0:1, :]),
                                             start=False, stop=True,
                                             skip_group_check=True)
                        for c in range(CC):
                            emit_tails(c)
                    else:
                        # steady state: conv c0+c1 on Pool, c2c3 as one
                        # wide ACT op; Ws preloads c0/c1 on ACT (ahead of
                        # the tanhs), c2/c3 on DVE; energy mult+sum on DVE
                        if pool_conv:
                            nc.gpsimd.tensor_copy(xbf[:, :E], x4[:, :E])
                            nc.gpsimd.tensor_copy(xbf[:, E:2 * E],
                                                  x4[:, E:2 * E])
                        else:
                            nc.scalar.copy(xbf[:, :E], x4[:, :E])
                            nc.scalar.copy(xbf[:, E:2 * E], x4[:, E:2 * E])
                        nc.scalar.copy(xbf[:, 2 * E:], x4[:, 2 * E:])
                        if preload_mode != "ones":
                            emit_pre(0, "act")
                            emit_pre(1, "act")

                        # tanh deferred from the previous iteration's c3:
                        # emitted after this block's conversion so the
                        # ACT queue doesn't convoy behind the PE's last
                        # matmul; its mult+sum after the c1 evacuation
                        if deferred_tail:
                            deferred_tail.pop(0)()
                        emit_T(0)
                        emit_evac(0)
                        emit_T(1)
                        emit_evac(1)
                        if deferred_ttr:
                            deferred_ttr.pop(0)()
                        emit_T(2)
                        emit_evac(2)
                        emit_T(3)
                        emit_evac(3)
                        if preload_mode == "ones":
                            for c in range(CC):
                                emit_M(c, preload="ones")
                                emit_tails(c)
                        else:
                            emit_M(0, preload="done")
                            emit_M(1, preload="done")
                            emit_M(2, preload="act")
                            emit_tails(0)
                            emit_M(3, preload="act")
                            emit_tails(1)
                            emit_tails(2)
                            emit_tails(3)

                    # epilogue for the previous batch goes after this
                    # sblk's PE stream so PE never waits on it
                    if sblk == 0 and pending_epi:
                        emit_epilogue(pending_epi)

                pending_epi.append((b, rep, energy_b))
                if b == B_SH - 1:
                    emit_epilogue(pending_epi)

    nc.compile()
    return nc


def shard_inputs(inputs):
    """Full inputs dict -> list of 8 per-core input dicts."""
    dec = np.ascontiguousarray(inputs["decoder_hidden"], dtype=np.float32)
    enc = np.ascontiguousarray(inputs["encoder_all_hidden"], dtype=np.float32)
    base = {
        "W_w": np.ascontiguousarray(inputs["W_w"], dtype=np.float32),
        "W_b": np.ascontiguousarray(inputs["W_b"], dtype=np.float32),
        "U_w": np.ascontiguousarray(inputs["U_w"], dtype=np.float32),
        "U_b": np.ascontiguousarray(inputs["U_b"], dtype=np.float32),
        "v_w": np.ascontiguousarray(inputs["v_w"], dtype=np.float32),
    }
    maps = []
    for c in range(N_CORES):
        m = dict(base)
        m["decoder_hidden"] = dec[c * B_SH:(c + 1) * B_SH]
        m["encoder_all_hidden"] = enc[c * B_SH:(c + 1) * B_SH]
        maps.append(m)
    return maps


_NC_CACHE = None


def get_program():
    global _NC_CACHE
    if _NC_CACHE is None:
        _NC_CACHE = build_program()
    return _NC_CACHE


def kernel(**inputs):
    from concourse import bass_utils
    nc = get_program()
    maps = shard_inputs(inputs)
    res = bass_utils.run_bass_kernel_spmd(nc, maps,
                                          core_ids=list(range(N_CORES)))
    return np.concatenate([res.results[c]["alpha"] for c in range(N_CORES)],
                          axis=0)
